# revision 4
# baseline (speedup 1.0000x reference)
"""Trainium2 Bass kernel for MoRAttention (sparse selective-KV GQA attention).

Math: the reference's argsort/gather of active keys == dense attention with
mask = active[k] & (pos[k] <= pos[q]) (softmax is permutation invariant).
We gather active keys on the HOST (x columns), so K/V projections and
attention run over SA = ceil(n_active/128)*128 compacted keys. Causal
structure over the sorted keys lets us statically skip dead (k-chunk,
q-span) tiles; only "straddle" tiles (partially-valid) get a multiplicative
mask, shipped precomputed from the host.

Sharding: 8 cores = 2 batches x 4 kv-groups. Core (b, g) computes q-heads
[4g, 4g+4) + kv-head g of batch b, producing a partial o_proj output
out^T [D, S]; the host sums the 4 partials per batch.

Everything in bf16 (matmul operands, DMA) with f32 PSUM accumulation:
PE rate is the same as f32r but DMA/SBUF/DVE cost halves and weight loads
are cheaper. Pipeline: K, V, Q0, B0|Q1, B1|Q2, B2|Q3, B3, C with attnV
staggered one k-chunk behind scores so PE never waits on exp.
"""

import numpy as np

S, D, HD = 1024, 2048, 128
NH = 4           # q heads per core
DC = D // 128    # contraction chunks
SCALE = HD ** -0.5

TRACE = False
LAST_EXEC_NS = None
LAST_RESULTS = None

_NC_CACHE = {}


def _build_nc(meta):
    import concourse.mybir as mybir
    from concourse import bacc
    from concourse.tile import TileContext
    from contextlib import ExitStack

    SAC, qa_kc, span01_kcs, mask_runs, nstr = meta
    SA = SAC * 128
    NSTR = max(1, nstr)
    # mask tile runs grouped by kc: (qt0, ntiles, idx0)
    runs_by_kc = {}
    for (kc, qt0, n, idx0) in mask_runs:
        runs_by_kc.setdefault(kc, []).append((qt0, n, idx0))

    f32 = mybir.dt.float32
    bf16 = mybir.dt.bfloat16
    Exp = mybir.ActivationFunctionType.Exp

    nc = bacc.Bacc("TRN2", target_bir_lowering=False, debug=False)

    xs_d = nc.dram_tensor("xs", [128, DC * S], bf16, kind="ExternalInput")
    xk_d = nc.dram_tensor("xk", [128, DC * SA], bf16, kind="ExternalInput")
    wq_d = nc.dram_tensor("wq", [128, DC * 512], bf16, kind="ExternalInput")
    wk_d = nc.dram_tensor("wk", [128, DC * 128], bf16, kind="ExternalInput")
    wv_d = nc.dram_tensor("wv", [128, DC * 128], bf16, kind="ExternalInput")
    wo_d = nc.dram_tensor("wo", [128, NH * D], bf16, kind="ExternalInput")
    cq_d = nc.dram_tensor("cq", [128, S], bf16, kind="ExternalInput")
    sq_d = nc.dram_tensor("sq", [128, S], bf16, kind="ExternalInput")
    ck_d = nc.dram_tensor("ck", [128, SA], bf16, kind="ExternalInput")
    sk_d = nc.dram_tensor("sk", [128, SA], bf16, kind="ExternalInput")
    mk_d = nc.dram_tensor("mk", [128, NSTR * 128], bf16, kind="ExternalInput")
    out_d = nc.dram_tensor("out", [128, DC * S], bf16, kind="ExternalOutput")

    with TileContext(nc) as tc, ExitStack() as ctx:
        singles = ctx.enter_context(tc.tile_pool(name="singles", bufs=1))
        persist = ctx.enter_context(tc.tile_pool(name="persist", bufs=1))

        ones_tmp = singles.tile([128, 128], f32)
        nc.vector.memset(ones_tmp, 1.0)
        ones128 = singles.tile([128, 128], bf16)
        nc.vector.tensor_copy(ones128, ones_tmp)

        # ---- resident SBUF tensors (host-prearranged layouts) ----
        xs_sb = persist.tile([128, DC * S], bf16, tag="xs")
        xk_sb = persist.tile([128, DC * SA], bf16, tag="xk")
        wq_sb = persist.tile([128, DC * 512], bf16, tag="wq")
        wk_sb = persist.tile([128, DC * 128], bf16, tag="wk")
        wv_sb = persist.tile([128, DC * 128], bf16, tag="wv")
        wo_sb = persist.tile([128, NH * D], bf16, tag="wo")
        cq_sb = persist.tile([128, S], bf16, tag="cq")
        sq_sb = persist.tile([128, S], bf16, tag="sq")
        ck_sb = persist.tile([128, SA], bf16, tag="ck")
        sk_sb = persist.tile([128, SA], bf16, tag="sk")
        mk_sb = persist.tile([128, NSTR * 128], bf16, tag="mk")

        kT = persist.tile([128, SA], bf16, tag="kT")
        vn = persist.tile([128, SA], bf16, tag="vn")
        qT = [persist.tile([128, S], bf16, tag=f"qT{h}", name=f"qT{h}") for h in range(NH)]
        attn = [persist.tile([128, S], bf16, tag=f"attn{h}", name=f"attn{h}") for h in range(NH)]

        # ---- DMA issue: three rings (sync=SP, scalar=Act, gpsimd=Pool) ----
        HK = DC * SA // 2
        # scalar ring: K/V weights + compact-x hi + rope tables + masks
        nc.scalar.dma_start(out=wk_sb, in_=wk_d[:, :])
        nc.scalar.dma_start(out=xk_sb[:, HK:], in_=xk_d[:, HK:])
        nc.scalar.dma_start(out=wv_sb, in_=wv_d[:, :])
        nc.scalar.dma_start(out=ck_sb, in_=ck_d[:, :])
        nc.scalar.dma_start(out=sk_sb, in_=sk_d[:, :])
        nc.scalar.dma_start(out=cq_sb, in_=cq_d[:, :])
        nc.scalar.dma_start(out=sq_sb, in_=sq_d[:, :])
        nc.scalar.dma_start(out=mk_sb, in_=mk_d[:, :])
        # sync ring: compact-x lo + q-proj weights (then swaps/wo/outs later)
        nc.sync.dma_start(out=xk_sb[:, :HK], in_=xk_d[:, :HK])
        for i in range(4):
            w = DC * 512 // 4
            nc.sync.dma_start(
                out=wq_sb[:, i * w:(i + 1) * w], in_=wq_d[:, i * w:(i + 1) * w]
            )
        # gpsimd (Pool, software DGE) ring: all 16 x chunks
        for dc in range(DC):
            nc.gpsimd.dma_start(
                out=xs_sb[:, dc * S:(dc + 1) * S], in_=xs_d[:, dc * S:(dc + 1) * S]
            )

        def rope(psum, cos_t, sin_t, dst, w, swaps_engine, pool):
            # dst = psum*cos + rot_half(psum)*sin2  (sin2 pre-arranged so a
            # plain half-swap after the multiply gives rot_half()*sin)
            pc = pool.tile([128, w], bf16, tag="ropec")
            ps_ = pool.tile([128, w], bf16, tag="ropes")
            pw = pool.tile([128, w], bf16, tag="ropew")
            nc.vector.tensor_mul(pc, psum, cos_t)
            nc.vector.tensor_mul(ps_, psum, sin_t)
            swaps_engine.dma_start(out=pw[0:64, :], in_=ps_[64:128, :])
            swaps_engine.dma_start(out=pw[64:128, :], in_=ps_[0:64, :])
            nc.vector.tensor_add(dst, pc, pw)

        # ================= Phase A: K, V, then Q heads =================
        with tc.tile_pool(name="pkv", bufs=2, space="PSUM") as pkv, \
             tc.tile_pool(name="ropep", bufs=2) as ropep:
            # K projection: kT_pre [HD, SA]
            psum_k = pkv.tile([128, SA], f32, tag="pkv")
            for dc in range(DC):
                nc.tensor.matmul(
                    psum_k[:, 0:512],
                    lhsT=wk_sb[:, dc * 128:(dc + 1) * 128],
                    rhs=xk_sb[:, dc * SA:dc * SA + 512],
                    start=(dc == 0), stop=(dc == DC - 1),
                )
            for dc in range(DC):
                nc.tensor.matmul(
                    psum_k[:, 512:SA],
                    lhsT=wk_sb[:, dc * 128:(dc + 1) * 128],
                    rhs=xk_sb[:, dc * SA + 512:(dc + 1) * SA],
                    start=(dc == 0), stop=(dc == DC - 1),
                )
            rope(psum_k, ck_sb, sk_sb, kT, SA, nc.sync, ropep)

            # V projection: vT [HD, SA] -> vn [SA-chunks, HD] via DMA transpose
            psum_v = pkv.tile([128, SA], f32, tag="pkv")
            for dc in range(DC):
                nc.tensor.matmul(
                    psum_v[:, 0:512],
                    lhsT=wv_sb[:, dc * 128:(dc + 1) * 128],
                    rhs=xk_sb[:, dc * SA:dc * SA + 512],
                    start=(dc == 0), stop=(dc == DC - 1),
                )
            for dc in range(DC):
                nc.tensor.matmul(
                    psum_v[:, 512:SA],
                    lhsT=wv_sb[:, dc * 128:(dc + 1) * 128],
                    rhs=xk_sb[:, dc * SA + 512:(dc + 1) * SA],
                    start=(dc == 0), stop=(dc == DC - 1),
                )
            vTe = ropep.tile([128, SA], bf16, tag="vTe")
            nc.scalar.copy(vTe, psum_v)
            for kc in range(SAC):
                nc.sync.dma_start(
                    out=vn[:, kc * 128:(kc + 1) * 128],
                    in_=vTe[:, kc * 128:(kc + 1) * 128],
                    transpose=True,
                )

        # ---------------- Q chains + attention, interleaved ----------------
        with tc.tile_pool(name="pq", bufs=2, space="PSUM") as pq, \
             tc.tile_pool(name="ropeq", bufs=2) as ropeq, \
             tc.tile_pool(name="ps", bufs=2, space="PSUM") as ps_p, \
             tc.tile_pool(name="po", bufs=1, space="PSUM") as po_p, \
             tc.tile_pool(name="pc", bufs=1, space="PSUM") as pc_p, \
             tc.tile_pool(name="ppool", bufs=2) as ppool, \
             tc.tile_pool(name="rpool", bufs=2) as rpool:

            def q_chain(h):
                # two 512-halves sequentially so rope of half 0 overlaps the
                # PE chain of half 1
                for qs in (0, 512):
                    psq = pq.tile([128, 512], f32, tag="pq")
                    for dc in range(DC):
                        nc.tensor.matmul(
                            psq,
                            lhsT=wq_sb[:, dc * 512 + h * 128: dc * 512 + (h + 1) * 128],
                            rhs=xs_sb[:, dc * S + qs: dc * S + qs + 512],
                            start=(dc == 0), stop=(dc == DC - 1),
                        )
                    rope(
                        psq, cq_sb[:, qs:qs + 512], sq_sb[:, qs:qs + 512],
                        qT[h][:, qs:qs + 512], 512, nc.sync, ropeq,
                    )

            def b_head(h):
                psum_o = po_p.tile([128, S], f32, tag="po")
                psum_c = pc_p.tile([128, S], f32, tag="pc")

                def spans(kc):
                    return [(0, 512), (512, 1024)] if qa_kc[kc] == 0 else [(512, 1024)]

                def scores_exp(kc):
                    p_sb = ppool.tile([128, S], bf16, tag="p_sb")
                    for (s0, s1) in spans(kc):
                        psum_s = ps_p.tile([128, 512], f32, tag="ps")
                        nc.tensor.matmul(
                            psum_s[:, 0:s1 - s0],
                            lhsT=kT[:, kc * 128:(kc + 1) * 128],
                            rhs=qT[h][:, s0:s1],
                            start=True, stop=True,
                        )
                        nc.scalar.activation(
                            p_sb[:, s0:s1], psum_s[:, 0:s1 - s0], Exp, scale=SCALE
                        )
                    for (qt0, n, idx0) in runs_by_kc.get(kc, ()):
                        nc.vector.tensor_mul(
                            p_sb[:, qt0 * 128:(qt0 + n) * 128],
                            p_sb[:, qt0 * 128:(qt0 + n) * 128],
                            mk_sb[:, idx0 * 128:(idx0 + n) * 128],
                        )
                    return p_sb

                def reduce_chunk(kc, p_sb):
                    for (s0, s1) in spans(kc):
                        if s0 == 0:
                            start = (kc == span01_kcs[0])
                            stop = (kc == span01_kcs[-1])
                        else:
                            start = (kc == 0)
                            stop = (kc == SAC - 1)
                        nc.tensor.matmul(
                            psum_c[:, s0:s1], lhsT=ones128,
                            rhs=p_sb[:, s0:s1], start=start, stop=stop,
                        )
                        nc.tensor.matmul(
                            psum_o[:, s0:s1],
                            lhsT=vn[:, kc * 128:(kc + 1) * 128],
                            rhs=p_sb[:, s0:s1], start=start, stop=stop,
                        )

                prev = None
                for kc in range(SAC):
                    p_sb = scores_exp(kc)
                    if prev is not None:
                        reduce_chunk(prev[0], prev[1])
                    prev = (kc, p_sb)
                reduce_chunk(prev[0], prev[1])

                rb = rpool.tile([128, S], f32, tag="rb")
                nc.vector.reciprocal_approx_fast(rb, psum_c)
                nc.vector.tensor_mul(attn[h], psum_o, rb)

            q_chain(0)
            for h in range(NH):
                b_head(h)
                if h + 1 < NH:
                    q_chain(h + 1)

        # load wo late on the sync ring (needed only by phase C)
        for i in range(2):
            w = NH * D // 2
            nc.sync.dma_start(
                out=wo_sb[:, i * w:(i + 1) * w], in_=wo_d[:, i * w:(i + 1) * w]
            )

        # ================= Phase C: out^T = wo^T @ attn =================
        with tc.tile_pool(name="poc", bufs=2, space="PSUM") as poc, \
             tc.tile_pool(name="outp", bufs=3) as outp:
            for dc in range(DC):
                oc = poc.tile([128, S], f32, tag="oc")
                for h in range(NH):
                    for qs in (0, 512):
                        nc.tensor.matmul(
                            oc[:, qs:qs + 512],
                            lhsT=wo_sb[:, h * D + dc * 128: h * D + (dc + 1) * 128],
                            rhs=attn[h][:, qs:qs + 512],
                            start=(h == 0), stop=(h == NH - 1),
                        )
                osb = outp.tile([128, S], bf16, tag="osb")
                if dc % 2 == 0:
                    nc.scalar.copy(osb, oc)
                else:
                    nc.vector.tensor_copy(osb, oc)
                nc.sync.dma_start(
                    out=out_d[:, dc * S:(dc + 1) * S], in_=osb
                )

    nc.compile()
    return nc


def _get_nc(meta):
    if meta not in _NC_CACHE:
        _NC_CACHE[meta] = _build_nc(meta)
    return _NC_CACHE[meta]


def _host_prep(hidden_states, cos, sin, wq, wk, wv, wo, position_ids, active_mask):
    import ml_dtypes

    bf16 = ml_dtypes.bfloat16
    hs = np.asarray(hidden_states, dtype=np.float32)
    cos = np.asarray(cos, dtype=np.float32)
    sin = np.asarray(sin, dtype=np.float32)
    wq = np.asarray(wq, dtype=np.float32)
    wk = np.asarray(wk, dtype=np.float32)
    wv = np.asarray(wv, dtype=np.float32)
    wo = np.asarray(wo, dtype=np.float32)
    pos = np.asarray(position_ids).astype(np.int64)
    am = np.asarray(active_mask).astype(bool)
    B = hs.shape[0]
    assert B == 2 and hs.shape[1] == S and hs.shape[2] == D

    ar = np.arange(S)
    sels, pos_sels, nacts = [], [], []
    for b in range(B):
        order = np.argsort(np.where(am[b], ar, ar + S), kind="stable")
        nact = int(am[b].sum())
        sel = order[:nact]
        sels.append(sel)
        pos_sels.append(pos[b][sel])
        nacts.append(nact)

    SAC = int(max((n + 127) // 128 for n in nacts))
    SA = SAC * 128

    # causal/active tile structure (union over batches so SPMD code is shared)
    live = np.zeros((SAC, 8), dtype=bool)
    full = np.ones((SAC, 8), dtype=bool)
    for b in range(B):
        ps = pos_sels[b]
        n = nacts[b]
        qmax = pos[b].reshape(8, 128).max(axis=1)
        qmin = pos[b].reshape(8, 128).min(axis=1)
        for kc in range(SAC):
            ks, ke = kc * 128, min(kc * 128 + 128, n)
            for qt in range(8):
                if ks >= n:
                    full[kc, qt] = False
                    continue
                l = ps[ks] <= qmax[qt]
                f = (ke - ks == 128) and (ps[ke - 1] <= qmin[qt])
                live[kc, qt] |= l
                if l and not f:
                    full[kc, qt] = False
                if not l:
                    full[kc, qt] = False

    qt_min = [int(np.argmax(live[kc])) if live[kc].any() else 8 for kc in range(SAC)]
    qa_kc = tuple(0 if q < 4 else 512 for q in qt_min)
    span01_kcs = tuple(kc for kc in range(SAC) if qa_kc[kc] == 0)

    mask_list = []
    for kc in range(SAC):
        for qt in range(qa_kc[kc] // 128, 8):
            if not full[kc, qt]:
                mask_list.append((kc, qt))
    # merge contiguous qt tiles of the same kc into runs
    mask_runs = []
    idx = 0
    i = 0
    while i < len(mask_list):
        kc, qt0 = mask_list[i]
        n = 1
        while (i + n < len(mask_list)
               and mask_list[i + n] == (kc, qt0 + n)):
            n += 1
        mask_runs.append((kc, qt0, n, idx))
        idx += n
        i += n
    mask_runs = tuple(mask_runs)
    meta = (SAC, qa_kc, span01_kcs, mask_runs, idx)
    NSTR = max(1, idx)

    s2 = np.concatenate([sin.T[64:], -sin.T[:64]], axis=0)  # [HD, S] table
    cq = cos.T.astype(bf16)
    sq = s2.astype(bf16)

    def chunked(a, nchunks):
        # [nchunks*128, F] -> [128, nchunks*F] with chunk c at cols [c*F, (c+1)*F)
        F = a.shape[1]
        return np.ascontiguousarray(
            a.reshape(nchunks, 128, F).transpose(1, 0, 2).reshape(128, nchunks * F)
        )

    in_maps = []
    for core in range(8):
        b, g = divmod(core, 4)
        n = nacts[b]
        ps = pos_sels[b]
        x = hs[b]                       # [S, D]
        xsel = np.zeros((SA, D), dtype=np.float32)
        xsel[:n] = x[sels[b]]

        ckb = np.zeros((128, SA), dtype=np.float32)
        skb = np.zeros((128, SA), dtype=np.float32)
        ckb[:, :n] = cos.T[:, ps]
        skb[:, :n] = s2[:, ps]

        mk = np.zeros((128, NSTR * 128), dtype=np.float32)
        for idx, (kc, qt) in enumerate(mask_list):
            ks = kc * 128
            kvalid = (ks + np.arange(128)) < n
            kp = ps[np.minimum(ks + np.arange(128), max(n - 1, 0))]
            qp = pos[b][qt * 128:(qt + 1) * 128]
            mk[:, idx * 128:(idx + 1) * 128] = (
                kvalid[:, None] & (kp[:, None] <= qp[None, :])
            ).astype(np.float32)

        in_maps.append({
            "xs": chunked(x.T.astype(bf16), DC),
            "xk": chunked(xsel.T.astype(bf16), DC),
            "wq": chunked(wq[:, g * 512:(g + 1) * 512].astype(bf16), DC),
            "wk": chunked(wk[:, g * 128:(g + 1) * 128].astype(bf16), DC),
            "wv": chunked(wv[:, g * 128:(g + 1) * 128].astype(bf16), DC),
            "wo": chunked(wo[g * 512:(g + 1) * 512].astype(bf16), NH),
            "cq": cq, "sq": sq,
            "ck": ckb.astype(bf16), "sk": skb.astype(bf16),
            "mk": mk.astype(bf16),
        })
    return meta, in_maps


def kernel(hidden_states, cos, sin, wq, wk, wv, wo, position_ids, active_mask):
    global LAST_EXEC_NS, LAST_RESULTS
    from concourse.bass_utils import run_bass_kernel_spmd

    meta, in_maps = _host_prep(
        hidden_states, cos, sin, wq, wk, wv, wo, position_ids, active_mask
    )
    nc = _get_nc(meta)
    res = run_bass_kernel_spmd(nc, in_maps, core_ids=list(range(8)), trace=TRACE)
    LAST_EXEC_NS = res.exec_time_ns
    LAST_RESULTS = res
    B = np.asarray(hidden_states).shape[0]
    full = np.zeros((B, S, D), dtype=np.float32)
    for core in range(8):
        b = core // 4
        o = np.asarray(res.results[core]["out"]).astype(np.float32)
        outT = o.reshape(128, DC, S).transpose(1, 0, 2).reshape(D, S)
        full[b] += outT.T
    return full
